# revision 27
# baseline (speedup 1.0000x reference)
"""Trainium2 Bass kernel for causal multi-head attention (B=4, T=2048, C=1024, H=16).

Sharding (8 cores, zero collectives): core c handles batch b=c//2 and head-half
half=c%2 (8 heads).  v2 design, engine-balanced:

  PE      only matmuls: QKV projections, S^T scores, P^T V, output projection.
  Act     only exp, on tightly-packed score tiles (no wasted columns).
  DVE     PSUM->SBUF copies for Q/K proj, reciprocal, causal-mask multiplies,
          y^T normalize multiplies.
  Pool    (gpsimd) V-proj copies, partition_broadcast of softmax reciprocal,
          output-projection copies/adds.
  DMA     column-sliced input loads ordered so head 0's dependencies (m-chunk 0
          of Wq/Wk, head-pair 0 of Wv, x tile 0) land first; output stores per
          128x512 tile as they finish.

Attention per head in S^T orientation ([key partitions, query free]):
  - score chunks packed tightly into [128, 1024] PSUM groups of 2 chunks
  - exp on ScalarE over the valid width only
  - causal mask applied AFTER exp as a 0/1 bf16 multiply on the 128-col
    diagonal region of each diagonal chunk (DVE, 2x mode)
  - P^T V accumulates into a [128,512] PSUM bank; V tiles store pairs
    [v_even | ones | v_odd] (129 cols) so even heads contract [v,ones]
    (rows 0..64) and odd heads [ones,v] (rows 63..127) -- softmax row-sums
    land in partition 64/63, y^T halves land in 0..63 / 64..127 with no
    cross-partition moves.
  - normalize: reciprocal (DVE) -> partition_broadcast (Pool) -> multiply
    (DVE) straight into the packed y^T tile.
Software pipelining: S(g+1) is emitted before PV(g) so the PE never waits on
exp; QKV projections for later tiles and earlier tiles' output projections
are dribbled between attention groups as PE filler (psw pool only -- the pss
score pipeline is never touched by fillers).  The last tile's output
projection contracts y^T chunks 0..2 during heads 6-7 and only chunk 3 in
the tail.
"""

import os
import sys

import numpy as np

for _p in ("/opt/trn_rl_repo", "/root/.axon_site/_ro/trn_rl_repo"):
    if os.path.isdir(_p) and _p not in sys.path:
        sys.path.insert(0, _p)

import ml_dtypes  # noqa: E402

import concourse.bass as bass  # noqa: E402
import concourse.bacc as bacc  # noqa: E402
import concourse.mybir as mybir  # noqa: E402
import concourse.tile as tile  # noqa: E402

BF16 = mybir.dt.bfloat16
F32 = mybir.dt.float32

C = 1024     # model dim
HALF = 512   # q/k/v columns per core (8 heads x 64)
HC = 8       # heads per core
D = 64       # head dim

_NC_CACHE: dict = {}


def _build_program(kc: int, T: int):
    """Single-core SPMD program.  kc = # of 128-row contraction chunks for the
    QKV projections (8, or 9 when biases are folded via an augmented row)."""
    nc = bacc.Bacc("TRN2", target_bir_lowering=False)

    xT = nc.dram_tensor("xT", [kc * 128, T], BF16, kind="ExternalInput")
    wq = nc.dram_tensor("wq", [kc * 128, HALF], BF16, kind="ExternalInput")
    wk = nc.dram_tensor("wk", [kc * 128, HALF], BF16, kind="ExternalInput")
    wv = nc.dram_tensor("wv", [kc * 128, HALF], BF16, kind="ExternalInput")
    wp = nc.dram_tensor("wp", [HALF, C], BF16, kind="ExternalInput")
    m01 = nc.dram_tensor("m01", [128, 128], BF16, kind="ExternalInput")
    # bf16 partials: halves output DMA traffic; host sums the two core
    # halves in f32.  Quantization adds ~0.3% RMS, well under the budget.
    outT = nc.dram_tensor("outT", [C, T], BF16, kind="ExternalOutput")

    nqt = T // 512    # number of 512-wide query tiles
    nkr = T // 128    # number of 128-row key chunks

    with tile.TileContext(nc) as tc:
        with (
            tc.tile_pool(name="const", bufs=1) as const,
            tc.tile_pool(name="pt", bufs=8) as ptp,
            tc.tile_pool(name="rnorm", bufs=3) as rnp,
            tc.tile_pool(name="outb", bufs=4) as obp,
            tc.tile_pool(name="ps_s", bufs=2, space="PSUM") as pss,
            tc.tile_pool(name="ps_w", bufs=1, space="PSUM") as psw,
            tc.tile_pool(name="ps_o", bufs=3, space="PSUM") as pso,
        ):
            xt_sb = const.tile([128, kc, T], BF16, tag="xt")
            wq_sb = const.tile([128, kc, HALF], BF16, tag="wq")
            wk_sb = const.tile([128, kc, HALF], BF16, tag="wk")
            wv_sb = const.tile([128, kc, HALF], BF16, tag="wv")
            wp_sb = const.tile([128, 4, C], BF16, tag="wp")
            m01_sb = const.tile([128, 128], BF16, tag="m01")
            kt_sb = const.tile([128, 4, T], BF16, tag="kt")
            qt_sb = const.tile([128, 4, T], BF16, tag="qt")
            # V tiles: per key chunk and head, [v | ones]
            vx_sb = const.tile([128, nkr, HC, 65], BF16, tag="vx")
            yt_sb = [const.tile([128, T], BF16, tag=f"yt{i}", name=f"yt{i}")
                     for i in range(HC // 2)]
            # f32 staging for the last tile's partial output projection
            oa_sb = const.tile([128, 8, 512], F32, tag="oa")

            # ---- input DMAs, ordered so head 0 can start ~6us in ----
            def dma_w(dst, src, mlo, mhi, eng=None):
                (eng or nc.sync).dma_start(
                    out=dst[:, :, 128 * mlo:128 * mhi],
                    in_=src[:, 128 * mlo:128 * mhi].rearrange(
                        "(k p) n -> p k n", p=128))

            def dma_x(n, klo, khi, eng=None):
                (eng or nc.sync).dma_start(
                    out=xt_sb[:, klo:khi, 512 * n:512 * n + 512],
                    in_=xT[128 * klo:128 * khi, 512 * n:512 * n + 512]
                    .rearrange("(k p) t -> p k t", p=128))

            kh = kc // 2
            nc.sync.dma_start(out=m01_sb[:], in_=m01[:, :])
            dma_w(wq_sb, wq, 0, 1)
            dma_x(0, 0, kh)
            dma_x(0, kh, kc)
            dma_w(wk_sb, wk, 0, 1)
            dma_w(wv_sb, wv, 0, 1)
            dma_w(wv_sb, wv, 1, 4)
            dma_w(wq_sb, wq, 1, 4)
            dma_w(wk_sb, wk, 1, 4)
            if nqt > 1:
                dma_x(1, 0, kh)
                dma_x(1, kh, kc)
            nc.sync.dma_start(
                out=wp_sb[:],
                in_=wp[:, :].rearrange("(k p) n -> p k n", p=128))
            for n in range(2, nqt):
                dma_x(n, 0, kh)
                dma_x(n, kh, kc)

            # ones column per head (col 64 of each 65 block)
            nc.vector.memset(vx_sb[:, :, :, 64:65], 1.0)
            # preload the exp activation table while DMAs run
            warm = rnp.tile([128, 1], BF16, tag="warm")
            nc.scalar.activation(out=warm[:], in_=m01_sb[:, 0:1],
                                 func=mybir.ActivationFunctionType.Exp)

            # ---- projection building blocks ----
            def proj_qk(w_sb, dst_sb, m, n, pool=None, tag=None):
                ps = (pool or psw).tile([128, 512], F32, tag=tag or "work")
                for k in range(kc):
                    nc.tensor.matmul(
                        ps[:, :],
                        w_sb[:, k, 128 * m:128 * m + 128],
                        xt_sb[:, k, 512 * n:512 * n + 512],
                        start=(k == 0), stop=(k == kc - 1))
                nc.vector.tensor_copy(
                    dst_sb[:, m, 512 * n:512 * n + 512], ps[:, :])

            def proj_v(kr, hlo=0, hhi=HC, pool=None, tag=None):
                """V projection for key chunk kr, heads hlo..hhi."""
                w = 64 * (hhi - hlo)
                ps = (pool or psw).tile([128, 512], F32, tag=tag or "work")
                for k in range(kc):
                    nc.tensor.matmul(
                        ps[:, 0:w],
                        xt_sb[:, k, 128 * kr:128 * kr + 128],
                        wv_sb[:, k, 64 * hlo:64 * hhi],
                        start=(k == 0), stop=(k == kc - 1))
                pv = ps[:, 0:w].rearrange("p (h e) -> p h e", e=64)
                nc.vector.tensor_copy(vx_sb[:, kr, hlo:hhi, 0:64], pv[:, :, :])

            def proj_out_m(qt, m):
                """Full output-projection m-chunk (contract all y^T chunks)."""
                ps = psw.tile([128, 512], F32, tag="work")
                for k in range(4):
                    nc.tensor.matmul(
                        ps[:, :],
                        wp_sb[:, k, 128 * m:128 * m + 128],
                        yt_sb[k][:, 512 * qt:512 * qt + 512],
                        start=(k == 0), stop=(k == 3))
                ob = obp.tile([128, 512], BF16, tag="ob")
                nc.vector.tensor_copy(ob[:], ps[:, :])
                nc.sync.dma_start(
                    out=outT[128 * m:128 * m + 128, 512 * qt:512 * qt + 512],
                    in_=ob[:])

            def proj_out_a(qt, m):
                """Partial (y^T chunks 0..2) into f32 staging."""
                ps = psw.tile([128, 512], F32, tag="work")
                for k in range(3):
                    nc.tensor.matmul(
                        ps[:, :],
                        wp_sb[:, k, 128 * m:128 * m + 128],
                        yt_sb[k][:, 512 * qt:512 * qt + 512],
                        start=(k == 0), stop=(k == 2))
                nc.vector.tensor_copy(oa_sb[:, m, :], ps[:, :])

            def proj_out_b(qt, m):
                """Final y^T chunk 3 + staged partial, store.  Alternates
                psw/pss (attention is over, pss banks are free) so
                consecutive units don't serialize on one PSUM bank."""
                if m % 2:
                    pst = pss.tile([128, 1024], F32, tag="smega", name="pob")
                    ps = pst[:, 0:512]
                else:
                    pst = psw.tile([128, 512], F32, tag="work", name="pob")
                    ps = pst[:, :]
                nc.tensor.matmul(
                    ps,
                    wp_sb[:, 3, 128 * m:128 * m + 128],
                    yt_sb[3][:, 512 * qt:512 * qt + 512],
                    start=True, stop=True)
                ob = obp.tile([128, 512], BF16, tag="ob")
                nc.vector.tensor_add(ob[:], ps, oa_sb[:, m, :])
                nc.sync.dma_start(
                    out=outT[128 * m:128 * m + 128, 512 * qt:512 * qt + 512],
                    in_=ob[:])

            # ---- attention ----
            def attention(h, qt):
                nch = 4 * qt + 4
                po = 64 * (h % 2)
                mch = h // 2
                ot = pso.tile([128, 512], F32, tag="o")

                chunks = []
                for j in range(nch):
                    dj = j - 4 * qt
                    qo = 128 * dj if dj > 0 else 0
                    chunks.append((j, qo, 512 - qo, dj >= 0))
                groups = []
                for g0 in range(0, nch, 2):
                    grp = chunks[g0:g0 + 2]
                    offs, off = [], 0
                    for (_, _, N, _) in grp:
                        offs.append(off)
                        off += N
                    groups.append((grp, offs, off))
                G = len(groups)

                sm_t = [None] * G
                pt_t = [None] * G

                def emit_S(g):
                    grp, offs, W = groups[g]
                    sm = pss.tile([128, 1024], F32, tag="smega")
                    sm_t[g] = sm
                    for (j, qo, N, _), off in zip(grp, offs):
                        nc.tensor.matmul(
                            sm[:, off:off + N],
                            kt_sb[po:po + 64, mch, 128 * j:128 * j + 128],
                            qt_sb[po:po + 64, mch,
                                  512 * qt + qo:512 * qt + 512],
                            start=True, stop=True)

                def emit_exp(g):
                    grp, offs, W = groups[g]
                    pt = ptp.tile([128, 1024], BF16, tag="pt")
                    pt_t[g] = pt
                    nc.scalar.activation(
                        out=pt[:, 0:W], in_=sm_t[g][:, 0:W],
                        func=mybir.ActivationFunctionType.Exp)
                    for (j, qo, N, masked), off in zip(grp, offs):
                        if masked:
                            # SBUF-only, so it can live on the Pool engine
                            nc.gpsimd.tensor_mul(
                                pt[:, off:off + 128],
                                pt[:, off:off + 128], m01_sb[:, :])

                def emit_PV(g):
                    grp, offs, W = groups[g]
                    pt = pt_t[g]
                    for (j, qo, N, _), off in zip(grp, offs):
                        nc.tensor.matmul(
                            ot[0:65, qo:qo + N],
                            vx_sb[:, j, h, 0:65],
                            pt[:, off:off + N],
                            start=(j == 0), stop=(j == nch - 1))

                def emit_norm():
                    rr = rnp.tile([1, 512], F32, tag="rr")
                    with nc.allow_low_precision(reason="softmax denom recip"):
                        nc.vector.reciprocal(rr[:], ot[64:65, :])
                    rb = rnp.tile([64, 512], F32, tag="rb")
                    nc.gpsimd.partition_broadcast(rb[:, :], rr[:, :])
                    if h % 2 == 0:
                        nc.vector.tensor_mul(
                            yt_sb[h // 2][0:64, 512 * qt:512 * qt + 512],
                            ot[0:64, :], rb[:, :])
                    else:
                        yto = rnp.tile([64, 512], BF16, tag="yto")
                        nc.vector.tensor_mul(yto[:], ot[0:64, :], rb[:, :])
                        nc.sync.dma_start(
                            out=yt_sb[h // 2][64:128,
                                              512 * qt:512 * qt + 512],
                            in_=yto[:])

                return emit_S, emit_exp, emit_PV, emit_norm, G

            # ---- schedule ----
            # fillers: list of (due, fn); due = processing position (qt, pos)
            # before which fn MUST be emitted (data read by that head), or
            # None for no deadline.  Dribbled front-first between groups.
            def run_head(h, qt, fillers, f_per_group):
                emit_S, emit_exp, emit_PV, emit_norm, G = attention(h, qt)
                debt = 0.0
                emit_S(0)
                for g in range(G):
                    if g + 1 < G:
                        emit_S(g + 1)
                    emit_exp(g)
                    debt += f_per_group
                    while debt >= 1.0 and fillers:
                        fillers.pop(0)[1]()
                        debt -= 1.0
                    emit_PV(g)
                emit_norm()

            # qt=0 bootstrap: h0/h1 need only m-chunk 0 of Q/K, heads 0-1 of V
            proj_qk(wq_sb, qt_sb, 0, 0, pool=pss, tag="smega")
            proj_qk(wk_sb, kt_sb, 0, 0, pool=pss, tag="smega")
            for kr in range(4):
                proj_v(kr, 0, 2, pool=(pss if kr % 2 else psw),
                       tag=("smega" if kr % 2 else None))

            # head processing order per tile; last tile ends on an even head
            # (no SBUF->SBUF DMA on the final norm chain)
            orders = [list(range(HC)) for _ in range(nqt)]
            orders[nqt - 1] = [0, 1, 2, 3, 4, 5, 7, 6]

            def pos_of(qt, h):
                return (qt, orders[qt].index(h))

            def qk_due(n, m):
                """Deadline for Q/K m-chunk of tile n: first head needing it."""
                if n >= nqt:
                    return None
                return min(pos_of(n, 2 * m), pos_of(n, 2 * m + 1))

            fillers = []   # persistent across tiles: (due, fn)
            for qt in range(nqt):
                if qt == 0:
                    # rest of V for key chunks 0-3 (heads 2-7), rest of Q/K m
                    for kr in range(4):
                        fillers.append(((0, 2),
                                        lambda kr=kr: proj_v(kr, 2, 8)))
                        if kr < 3:
                            m = kr + 1
                            fillers.append((qk_due(0, m),
                                            lambda m=m: proj_qk(wq_sb, qt_sb, m, 0)))
                            fillers.append((qk_due(0, m),
                                            lambda m=m: proj_qk(wk_sb, kt_sb, m, 0)))
                    if nqt > 1:
                        for m in range(4):
                            fillers.append((qk_due(1, m),
                                            lambda m=m: proj_qk(wq_sb, qt_sb, m, 1)))
                        for m in range(4):
                            fillers.append((qk_due(1, m),
                                            lambda m=m: proj_qk(wk_sb, kt_sb, m, 1)))
                        for kr in range(4, 8):
                            fillers.append(((1, 0), lambda kr=kr: proj_v(kr)))
                elif qt < nqt - 1:
                    n = qt + 1
                    for m in range(4):
                        fillers.append((qk_due(n, m),
                                        lambda m=m, n=n: proj_qk(wq_sb, qt_sb, m, n)))
                    for m in range(4):
                        fillers.append((qk_due(n, m),
                                        lambda m=m, n=n: proj_qk(wk_sb, kt_sb, m, n)))
                    for kr in range(4 * n, 4 * n + 4):
                        fillers.append(((n, 0), lambda kr=kr: proj_v(kr)))
                    if qt == nqt - 2:
                        for m in range(8):
                            fillers.append((None,
                                            lambda m=m, qt=qt: proj_out_m(qt - 2, m)))
                else:
                    # both remaining earlier tiles' output projections dribble
                    # here -- the last tile is the Act-bound one
                    for m in range(8):
                        fillers.append((None,
                                        lambda m=m, qt=qt: proj_out_m(qt - 2, m)))
                    for m in range(8):
                        fillers.append((None,
                                        lambda m=m, qt=qt: proj_out_m(qt - 1, m)))

                for i, h in enumerate(orders[qt]):
                    # force-emit anything this head's attention reads
                    overdue = [fn for (d, fn) in fillers
                               if d is not None and d <= (qt, i)]
                    if overdue:
                        fillers[:] = [(d, fn) for (d, fn) in fillers
                                      if not (d is not None and d <= (qt, i))]
                        for fn in overdue:
                            fn()
                    groups_left = (HC - i) * (2 * qt + 2)
                    # reserve ~2 units to cover the final norm chain
                    avail = len(fillers) - (2 if i == HC - 1 else 0)
                    f = max(avail, 0) / groups_left
                    run_head(h, qt, fillers, f)
                    if qt == nqt - 1 and i == 5:
                        for m in range(8):
                            fillers.append((None, lambda m=m: proj_out_a(qt, m)))
                if qt == nqt - 1:
                    while fillers:
                        fillers.pop(0)[1]()
                    for m in range(8):
                        proj_out_b(qt, m)

    nc.finalize()
    return nc


def _prep_inputs(x, Wq, bq, Wk, bk, Wv, bv, Wp, bp, T):
    """Builds per-core in_maps.  Returns (in_maps, kc, use_bias)."""
    bf = ml_dtypes.bfloat16
    scale = 1.0 / np.sqrt(D)
    use_bias = bool(np.any(bq) or np.any(bk) or np.any(bv))
    kc = 9 if use_bias else 8

    # S^T orientation: partition = key-in-chunk kk, free = query offset qq.
    # Valid (unmasked) iff qq >= kk.
    m01_np = (np.arange(128)[None, :] >= np.arange(128)[:, None]).astype(
        np.float32).astype(bf)

    def aug_x(xt):  # [1024, T] -> [kc*128, T]
        if not use_bias:
            return xt
        pad = np.zeros((128, xt.shape[1]), dtype=xt.dtype)
        pad[0, :] = 1.0
        return np.concatenate([xt, pad], axis=0)

    def aug_w(w, b):  # [1024, 512] -> [kc*128, 512]
        if not use_bias:
            return w
        pad = np.zeros((128, w.shape[1]), dtype=w.dtype)
        pad[0, :] = b
        return np.concatenate([w, pad], axis=0)

    in_maps = []
    for core in range(8):
        b = core // 2
        half = core % 2
        cs = slice(HALF * half, HALF * half + HALF)
        xt = np.ascontiguousarray(x[b, :T, :].T).astype(np.float32)
        in_maps.append({
            "xT": aug_x(xt).astype(bf),
            "wq": aug_w(Wq[:, cs] * scale, bq[cs] * scale).astype(bf),
            "wk": aug_w(Wk[:, cs], bk[cs]).astype(bf),
            "wv": aug_w(Wv[:, cs], bv[cs]).astype(bf),
            "wp": Wp[cs, :].astype(bf),
            "m01": m01_np,
        })
    return in_maps, kc, use_bias


def run(inputs: dict, T: int = 2048, trace: bool = False, tmpdir=None):
    """Returns (output [B,T,C] f32, BassKernelResults)."""
    from concourse.bass_utils import run_bass_kernel_spmd

    x = np.asarray(inputs["x"], dtype=np.float32)
    B = x.shape[0]
    in_maps, kc, _ = _prep_inputs(
        x, *[np.asarray(inputs[k], dtype=np.float32) for k in
             ("Wq", "bq", "Wk", "bk", "Wv", "bv", "Wp", "bp")], T)

    key = (kc, T)
    if key not in _NC_CACHE:
        _NC_CACHE[key] = _build_program(kc, T)
    nc = _NC_CACHE[key]

    res = run_bass_kernel_spmd(nc, in_maps, list(range(8)),
                               trace=trace, tmpdir=tmpdir)

    bp = np.asarray(inputs["bp"], dtype=np.float32)
    out = np.empty((B, T, C), dtype=np.float32)
    for b in range(B):
        acc = (res.results[2 * b]["outT"].astype(np.float32)
               + res.results[2 * b + 1]["outT"].astype(np.float32))
        out[b] = acc.T + bp[None, :]
    return out, res


def kernel(**inputs) -> np.ndarray:
    out, _ = run(inputs, T=2048, trace=False)
    return out


# revision 70
# speedup vs baseline: 2.2096x; 2.2096x over previous
"""Trainium2 Bass kernel for causal multi-head attention (B=4, T=2048, C=1024, H=16).

Sharding (8 cores, zero collectives): core c handles batch b=c//2 and head-half
half=c%2 (8 heads).  v2 design, engine-balanced:

  PE      only matmuls: QKV projections, S^T scores (with the causal mask
          folded in as an eye^T @ (-1e30 mask) PSUM accumulate), P^T V,
          output projection.
  Act     exp on tightly-packed score tiles (no wasted columns), plus
          PSUM->SBUF copies for the early tiles' Q/K/V (exp-idle phase).
  DVE     remaining PSUM->SBUF copies, reciprocal, normalize multiplies.
  Pool    (gpsimd; cannot touch PSUM) partition_broadcast of the softmax
          reciprocal.
  DMA     column-sliced input loads ordered so head 0's dependencies (m-chunk
          0 of Wq/Wk/Wv, x tile 0) land first; bf16 output partials stored
          per 128x512 tile as they finish (host sums the two halves in f32).

Attention per head in S^T orientation ([key partitions, query free]):
  - score chunks packed tightly into [128, 1024] PSUM groups of 2 chunks;
    diagonal chunks accumulate the additive causal mask on the PE so exp
    underflows masked entries to zero with no cross-engine hop
  - exp on ScalarE over the valid width only
  - P^T V accumulates into a [128,512] PSUM bank; V tiles are [v | ones]
    (65 cols) so softmax row-sums land in partition 64 for free
  - normalize: reciprocal (DVE) -> partition_broadcast (Pool) -> multiply
    (DVE); odd heads' halves move to y^T partitions 64-127 via an
    SBUF->SBUF DMA (PE output base partitions must be 0/32/64).
Software pipelining: S(g+1) is emitted before PV(g) so the PE never waits on
exp; QKV projections for later tiles and earlier tiles' output projections
are dribbled between attention groups as PE filler (psw pool only -- the pss
score pipeline is never touched by fillers) under (head, group)-granular due
dates that place each unit at the latest point before its first reader.  The
last tile's output projection contracts y^T chunks 0..2 during heads 7/6 and
only chunk 3 in the tail, alternating psw/pss banks.
"""

import os
import sys

import numpy as np

for _p in ("/opt/trn_rl_repo", "/root/.axon_site/_ro/trn_rl_repo"):
    if os.path.isdir(_p) and _p not in sys.path:
        sys.path.insert(0, _p)

import ml_dtypes  # noqa: E402

import concourse.bass as bass  # noqa: E402
import concourse.bacc as bacc  # noqa: E402
import concourse.mybir as mybir  # noqa: E402
import concourse.tile as tile  # noqa: E402

BF16 = mybir.dt.bfloat16
F32 = mybir.dt.float32

C = 1024     # model dim
HALF = 512   # q/k/v columns per core (8 heads x 64)
HC = 8       # heads per core
D = 64       # head dim

_NC_CACHE: dict = {}


def _build_program(kc: int, T: int):
    """Single-core SPMD program.  kc = # of 128-row contraction chunks for the
    QKV projections (8, or 9 when biases are folded via an augmented row)."""
    nc = bacc.Bacc("TRN2", target_bir_lowering=False)

    xT = nc.dram_tensor("xT", [kc * 128, T], BF16, kind="ExternalInput")
    # host pre-transposes to [m-piece, partition, k*128] so every DMA run is
    # 2KB+ on both sides (sub-512B descriptors pay 2x on the DMA engines)
    wq = nc.dram_tensor("wq", [4, 128, kc * 128], BF16, kind="ExternalInput")
    wk = nc.dram_tensor("wk", [4, 128, kc * 128], BF16, kind="ExternalInput")
    wv = nc.dram_tensor("wv", [4, 128, kc * 128], BF16, kind="ExternalInput")
    wp = nc.dram_tensor("wp", [HALF, C], BF16, kind="ExternalInput")
    mka = nc.dram_tensor("mka", [128, 128], BF16, kind="ExternalInput")
    eye = nc.dram_tensor("eye", [128, 128], BF16, kind="ExternalInput")
    # bf16 partials: halves output DMA traffic; host sums the two core
    # halves in f32.  Quantization adds ~0.3% RMS, well under the budget.
    outT = nc.dram_tensor("outT", [C, T], BF16, kind="ExternalOutput")

    nqt = T // 512    # number of 512-wide query tiles
    nkr = T // 128    # number of 128-row key chunks

    with tile.TileContext(nc) as tc:
        with (
            tc.tile_pool(name="const", bufs=1) as const,
            tc.tile_pool(name="pt", bufs=8) as ptp,
            tc.tile_pool(name="rnorm", bufs=3) as rnp,
            tc.tile_pool(name="outb", bufs=4) as obp,
            tc.tile_pool(name="ps_s", bufs=2, space="PSUM") as pss,
            tc.tile_pool(name="ps_w", bufs=2, space="PSUM") as psw,
            tc.tile_pool(name="ps_o", bufs=2, space="PSUM") as pso,
        ):
            xt_sb = const.tile([128, kc, T], BF16, tag="xt")
            wq_sb = const.tile([128, 4, kc, 128], BF16, tag="wq")
            wk_sb = const.tile([128, 4, kc, 128], BF16, tag="wk")
            wv_sb = const.tile([128, 4, kc, 128], BF16, tag="wv")
            wp_sb = const.tile([128, 4, C], BF16, tag="wp")
            mka_sb = const.tile([128, 128], BF16, tag="mka")
            eye_sb = const.tile([128, 128], BF16, tag="eye")
            kt_sb = const.tile([128, 4, T], BF16, tag="kt")
            qt_sb = const.tile([128, 4, T], BF16, tag="qt")
            # V tiles: per key chunk and head, [v | ones]
            vx_sb = const.tile([128, nkr, HC, 65], BF16, tag="vx")
            yt_sb = [const.tile([128, T], BF16, tag=f"yt{i}", name=f"yt{i}")
                     for i in range(HC // 2)]
            # f32 staging for the last tile's partial output projection
            oa_sb = const.tile([128, 8, 512], F32, tag="oa")

            # ---- input DMAs, ordered so head 0 can start ~6us in ----
            def dma_w(dst, src, mlo, mhi, eng=None):
                (eng or nc.sync).dma_start(
                    out=dst[:, mlo:mhi, :, :],
                    in_=src[mlo:mhi, :, :].rearrange(
                        "m p (k c) -> p m k c", k=kc))

            def dma_x(n, klo, khi, eng=None):
                (eng or nc.sync).dma_start(
                    out=xt_sb[:, klo:khi, 512 * n:512 * n + 512],
                    in_=xT[128 * klo:128 * khi, 512 * n:512 * n + 512]
                    .rearrange("(k p) t -> p k t", p=128))

            kh = kc // 2
            dma_w(wq_sb, wq, 0, 1)
            dma_x(0, 0, kh)
            dma_x(0, kh, kc)
            dma_w(wk_sb, wk, 0, 1)
            dma_w(wv_sb, wv, 0, 1)
            nc.sync.dma_start(out=mka_sb[:], in_=mka[:, :])
            nc.sync.dma_start(out=eye_sb[:], in_=eye[:, :])
            dma_w(wv_sb, wv, 1, 4)
            dma_w(wq_sb, wq, 1, 4)
            dma_w(wk_sb, wk, 1, 4)
            if nqt > 1:
                dma_x(1, 0, kh)
                dma_x(1, kh, kc)
            nc.sync.dma_start(
                out=wp_sb[:],
                in_=wp[:, :].rearrange("(k p) n -> p k n", p=128))
            for n in range(2, nqt):
                dma_x(n, 0, kh)
                dma_x(n, kh, kc)

            # ones column per head (col 64 of each 65 block)
            nc.vector.memset(vx_sb[:, :, :, 64:65], 1.0)
            # preload the exp activation table while DMAs run
            warm = rnp.tile([128, 1], BF16, tag="warm")
            nc.scalar.activation(out=warm[:], in_=eye_sb[:, 0:1],
                                 func=mybir.ActivationFunctionType.Exp)
            # PE p-state warmup: dummy matmuls on m01 while the x tile loads;
            # by the time real projections start the PE runs at full clock
            dmy = pso.tile([128, 128], F32, tag="o", name="dmy")
            for _ in range(int(os.environ.get("WARMUP_N", "16"))):
                nc.tensor.matmul(dmy[:, :], m01_sb[:, :], m01_sb[:, :],
                                 start=True, stop=True)

            # ---- projection building blocks ----
            def proj_qk(w_sb, dst_sb, m, n, pool=None, tag=None):
                ps = (pool or psw).tile([128, 512], F32, tag=tag or "work")
                for k in range(kc):
                    nc.tensor.matmul(
                        ps[:, :],
                        w_sb[:, m, k, :],
                        xt_sb[:, k, 512 * n:512 * n + 512],
                        start=(k == 0), stop=(k == kc - 1))
                if n <= 1:
                    # early tiles: ScalarE is exp-idle, take copies off DVE
                    nc.scalar.copy(
                        dst_sb[:, m, 512 * n:512 * n + 512], ps[:, :])
                else:
                    nc.vector.tensor_copy(
                        dst_sb[:, m, 512 * n:512 * n + 512], ps[:, :])

            def proj_v(kr, hlo=0, hhi=HC, pool=None, tag=None):
                """V projection for key chunk kr, heads hlo..hhi."""
                w = 64 * (hhi - hlo)
                ps = (pool or psw).tile([128, 512], F32, tag=tag or "work")
                for k in range(kc):
                    nc.tensor.matmul(
                        ps[:, 0:w],
                        xt_sb[:, k, 128 * kr:128 * kr + 128],
                        wv_sb[:, hlo // 2:hhi // 2, k, :],
                        start=(k == 0), stop=(k == kc - 1))
                pv = ps[:, 0:w].rearrange("p (h e) -> p h e", e=64)
                if kr < 8:
                    # early key chunks: ScalarE is exp-idle then
                    nc.scalar.copy(vx_sb[:, kr, hlo:hhi, 0:64], pv[:, :, :])
                else:
                    nc.vector.tensor_copy(
                        vx_sb[:, kr, hlo:hhi, 0:64], pv[:, :, :])

            def proj_out_m(qt, m):
                """Full output-projection m-chunk (contract all y^T chunks)."""
                ps = psw.tile([128, 512], F32, tag="work")
                for k in range(4):
                    nc.tensor.matmul(
                        ps[:, :],
                        wp_sb[:, k, 128 * m:128 * m + 128],
                        yt_sb[k][:, 512 * qt:512 * qt + 512],
                        start=(k == 0), stop=(k == 3))
                ob = obp.tile([128, 512], BF16, tag="ob")
                nc.vector.tensor_copy(ob[:], ps[:, :])
                nc.sync.dma_start(
                    out=outT[128 * m:128 * m + 128, 512 * qt:512 * qt + 512],
                    in_=ob[:])

            def proj_out_a(qt, m):
                """Partial (y^T chunks 0..2).  Even m stages in f32 for the
                tail's DVE add; odd m goes straight out to outT (host adds
                outB for those rows)."""
                ps = psw.tile([128, 512], F32, tag="work")
                for k in range(3):
                    nc.tensor.matmul(
                        ps[:, :],
                        wp_sb[:, k, 128 * m:128 * m + 128],
                        yt_sb[k][:, 512 * qt:512 * qt + 512],
                        start=(k == 0), stop=(k == 2))
                nc.vector.tensor_copy(oa_sb[:, m, :], ps[:, :])

            def proj_out_b(qt, m):
                """Final y^T chunk 3 + staged partial, store.  Alternates
                psw/pss banks AND the reading engine (DVE add for even m,
                Act copy + host add for odd m) so the tail isn't serialized
                on one PSUM bank or one engine."""
                if m % 2:
                    pst = pss.tile([128, 1024], F32, tag="smega", name="pob")
                else:
                    pst = psw.tile([128, 512], F32, tag="work", name="pob")
                ps = pst[:, 0:512]
                nc.tensor.matmul(
                    ps,
                    wp_sb[:, 3, 128 * m:128 * m + 128],
                    yt_sb[3][:, 512 * qt:512 * qt + 512],
                    start=True, stop=True)
                ob = obp.tile([128, 512], BF16, tag="ob")
                nc.vector.tensor_add(ob[:], ps, oa_sb[:, m, :])
                nc.sync.dma_start(
                    out=outT[128 * m:128 * m + 128, 512 * qt:512 * qt + 512],
                    in_=ob[:])

            # ---- attention ----
            def attention(h, qt):
                nch = 4 * qt + 4
                po = 64 * (h % 2)
                mch = h // 2
                ot = pso.tile([128, 512], F32, tag="o")

                chunks = []
                for j in range(nch):
                    dj = j - 4 * qt
                    qo = 128 * dj if dj > 0 else 0
                    chunks.append((j, qo, 512 - qo, dj >= 0))
                groups = []
                for g0 in range(0, nch, 2):
                    grp = chunks[g0:g0 + 2]
                    offs, off = [], 0
                    for (_, _, N, _) in grp:
                        offs.append(off)
                        off += N
                    groups.append((grp, offs, off))
                G = len(groups)

                sm_t = [None] * G
                pt_t = [None] * G

                def emit_S(g):
                    grp, offs, W = groups[g]
                    sm = pss.tile([128, 1024], F32, tag="smega")
                    sm_t[g] = sm
                    for (j, qo, N, masked), off in zip(grp, offs):
                        nc.tensor.matmul(
                            sm[:, off:off + N],
                            kt_sb[po:po + 64, mch, 128 * j:128 * j + 128],
                            qt_sb[po:po + 64, mch,
                                  512 * qt + qo:512 * qt + 512],
                            start=True, stop=not masked)
                        if masked:
                            # S += eye^T @ mka on the triangle columns; exp of
                            # -1e30 underflows to 0, so P leaves exp pre-masked
                            nc.tensor.matmul(
                                sm[:, off:off + 128],
                                eye_sb[:, :], mka_sb[:, :],
                                start=False, stop=True)

                def emit_exp(g):
                    grp, offs, W = groups[g]
                    pt = ptp.tile([128, 1024], BF16, tag="pt")
                    pt_t[g] = pt
                    nc.scalar.activation(
                        out=pt[:, 0:W], in_=sm_t[g][:, 0:W],
                        func=mybir.ActivationFunctionType.Exp)

                def emit_PV(g):
                    grp, offs, W = groups[g]
                    pt = pt_t[g]
                    for (j, qo, N, _), off in zip(grp, offs):
                        nc.tensor.matmul(
                            ot[0:65, qo:qo + N],
                            vx_sb[:, j, h, 0:65],
                            pt[:, off:off + N],
                            start=(j == 0), stop=(j == nch - 1))

                def emit_norm():
                    rr = rnp.tile([1, 512], F32, tag="rr")
                    with nc.allow_low_precision(reason="softmax denom recip"):
                        nc.vector.reciprocal(rr[:], ot[64:65, :])
                    rb = rnp.tile([64, 512], F32, tag="rb")
                    nc.gpsimd.partition_broadcast(rb[:, :], rr[:, :])
                    if h % 2 == 0:
                        nc.vector.tensor_mul(
                            yt_sb[h // 2][0:64, 512 * qt:512 * qt + 512],
                            ot[0:64, :], rb[:, :])
                    else:
                        yto = rnp.tile([64, 512], BF16, tag="yto")
                        nc.vector.tensor_mul(yto[:], ot[0:64, :], rb[:, :])
                        nc.sync.dma_start(
                            out=yt_sb[h // 2][64:128,
                                              512 * qt:512 * qt + 512],
                            in_=yto[:])

                return emit_S, emit_exp, emit_PV, emit_norm, G

            # ---- schedule ----
            # fillers: list of (hdue, gdue, fn).  hdue = (qt, pos) processing
            # position whose attention first reads fn's output (None = no
            # deadline); gdue = first reading group within that head (-1 =
            # needed before the head starts).  Dribbled front-first between
            # groups; forced at the latest legal point otherwise.
            def run_head(h, qt, i, fillers, f_per_group):
                emit_S, emit_exp, emit_PV, emit_norm, G = attention(h, qt)
                here = (qt, i)

                def force(g):
                    due = [fn for (hd, gd, fn) in fillers
                           if hd == here and gd <= g]
                    if due:
                        fillers[:] = [x for x in fillers
                                      if not (x[0] == here and x[1] <= g)]
                        for fn in due:
                            fn()

                debt = 0.0
                emit_S(0)
                for g in range(G):
                    if g + 1 < G:
                        emit_S(g + 1)
                    emit_exp(g)
                    force(g)
                    debt += f_per_group
                    while debt >= 1.0 and fillers:
                        fillers.pop(0)[2]()
                        debt -= 1.0
                    emit_PV(g)
                emit_norm()

            # qt=0 bootstrap: h0/h1 need only m-chunk 0 of Q/K, heads 0-1 of V
            proj_qk(wq_sb, qt_sb, 0, 0, pool=pss, tag="smega")
            proj_qk(wk_sb, kt_sb, 0, 0, pool=pss, tag="smega")
            for kr in range(4):
                proj_v(kr, 0, 2, pool=(pss if kr % 2 else psw),
                       tag=("smega" if kr % 2 else None))

            # head processing order per tile; last tile ends on an even head
            # (no SBUF->SBUF DMA on the final norm chain)
            orders = [list(range(HC)) for _ in range(nqt)]
            orders[nqt - 1] = [0, 1, 2, 3, 4, 5, 7, 6]

            def pos_of(qt, h):
                return (qt, orders[qt].index(h))

            def qk_due(n, m):
                """Deadline for Q/K m-chunk of tile n: first head needing it."""
                if n >= nqt:
                    return None
                return min(pos_of(n, 2 * m), pos_of(n, 2 * m + 1))

            fillers = []   # persistent across tiles: (hdue, gdue, fn)

            def halves(unit_fn, hdue, gdue):
                """One dribble entry per projection unit (half-splitting the
                k-chains measured slower in the timeline sim)."""
                cell = {}
                fillers.append((hdue, gdue,
                                lambda: (unit_fn(cell, 0), unit_fn(cell, 1))))

            def qk_unit(w_sb, dst_sb, m, n):
                def fn(cell, half):
                    if half == 0:
                        cell["ps"] = psw.tile([128, 512], F32, tag="work",
                                              name="ps")
                    ps = cell["ps"]
                    lo = 0 if half == 0 else kc // 2
                    hi = kc // 2 if half == 0 else kc
                    for k in range(lo, hi):
                        nc.tensor.matmul(
                            ps[:, :],
                            w_sb[:, m, k, :],
                            xt_sb[:, k, 512 * n:512 * n + 512],
                            start=(k == 0), stop=(k == kc - 1))
                    if half == 1:
                        if n <= 1:
                            nc.scalar.copy(
                                dst_sb[:, m, 512 * n:512 * n + 512], ps[:, :])
                        else:
                            nc.vector.tensor_copy(
                                dst_sb[:, m, 512 * n:512 * n + 512], ps[:, :])
                return fn

            def v_unit(kr):
                def fn(cell, half):
                    if half == 0:
                        cell["ps"] = psw.tile([128, 512], F32, tag="work",
                                              name="ps")
                    ps = cell["ps"]
                    lo = 0 if half == 0 else kc // 2
                    hi = kc // 2 if half == 0 else kc
                    for k in range(lo, hi):
                        nc.tensor.matmul(
                            ps[:, :],
                            xt_sb[:, k, 128 * kr:128 * kr + 128],
                            wv_sb[:, :, k, :],
                            start=(k == 0), stop=(k == kc - 1))
                    if half == 1:
                        pv = ps[:, :].rearrange("p (h e) -> p h e", e=64)
                        if kr < 8:
                            nc.scalar.copy(vx_sb[:, kr, :, 0:64], pv[:, :, :])
                        else:
                            nc.vector.tensor_copy(
                                vx_sb[:, kr, :, 0:64], pv[:, :, :])
                return fn

            def add_qkv_fillers(n):
                """Projections for tile n, with latest-legal deadlines."""
                for m in range(4):
                    halves(qk_unit(wq_sb, qt_sb, m, n), qk_due(n, m), -1)
                for m in range(4):
                    halves(qk_unit(wk_sb, kt_sb, m, n), qk_due(n, m),
                           max(0, 2 * n - 2))
                for kr in range(4 * n, 4 * n + 4):
                    halves(v_unit(kr), (n, 0), kr // 2)

            def po_unit(oqt, m):
                def fn(cell, half):
                    if half == 0:
                        cell["ps"] = psw.tile([128, 512], F32, tag="work",
                                              name="ps")
                    ps = cell["ps"]
                    for k in ((0, 1) if half == 0 else (2, 3)):
                        nc.tensor.matmul(
                            ps[:, :],
                            wp_sb[:, k, 128 * m:128 * m + 128],
                            yt_sb[k][:, 512 * oqt:512 * oqt + 512],
                            start=(k == 0), stop=(k == 3))
                    if half == 1:
                        ob = obp.tile([128, 512], BF16, tag="ob")
                        nc.vector.tensor_copy(ob[:], ps[:, :])
                        nc.sync.dma_start(
                            out=outT[128 * m:128 * m + 128,
                                     512 * oqt:512 * oqt + 512],
                            in_=ob[:])
                return fn

            for qt in range(nqt):
                if qt == 0:
                    # rest of V for key chunks 0-3 (heads 2-7), rest of Q/K m
                    for kr in range(4):
                        fillers.append(((0, 2), kr // 2,
                                        lambda kr=kr: proj_v(kr, 2, 8)))
                        if kr < 3:
                            m = kr + 1
                            fillers.append((qk_due(0, m), -1,
                                            lambda m=m: proj_qk(wq_sb, qt_sb, m, 0)))
                            fillers.append((qk_due(0, m), -1,
                                            lambda m=m: proj_qk(wk_sb, kt_sb, m, 0)))
                    if nqt > 1:
                        add_qkv_fillers(1)
                elif qt < nqt - 1:
                    add_qkv_fillers(qt + 1)
                    if qt == nqt - 2:
                        for m in range(8):
                            halves(po_unit(qt - 2, m), None, 0)
                else:
                    # both remaining earlier tiles' output projections dribble
                    # here -- the last tile is the Act-bound one
                    for m in range(8):
                        halves(po_unit(qt - 2, m), None, 0)
                    for m in range(8):
                        halves(po_unit(qt - 1, m), None, 0)

                for i, h in enumerate(orders[qt]):
                    # force-emit anything read before this head's groups
                    overdue = [fn for (hd, gd, fn) in fillers
                               if hd is not None
                               and (hd < (qt, i) or (hd == (qt, i) and gd < 0))]
                    if overdue:
                        fillers[:] = [
                            x for x in fillers
                            if not (x[0] is not None
                                    and (x[0] < (qt, i)
                                         or (x[0] == (qt, i) and x[1] < 0)))]
                        for fn in overdue:
                            fn()
                    groups_left = (HC - i) * (2 * qt + 2)
                    # reserve ~2 units to cover the final norm chain
                    avail = len(fillers) - (1 if i == HC - 1 else 0)
                    f = 0.85 * max(avail, 0) / groups_left
                    run_head(h, qt, i, fillers, f)
                    if qt == nqt - 1 and i == 5:
                        for m in range(8):
                            fillers.append((None, 0,
                                            lambda m=m: proj_out_a(qt, m)))
                if qt == nqt - 1:
                    while fillers:
                        fillers.pop(0)[2]()
                    for m in range(8):
                        proj_out_b(qt, m)

    nc.finalize()
    return nc


def _prep_inputs(x, Wq, bq, Wk, bk, Wv, bv, Wp, bp, T):
    """Builds per-core in_maps.  Returns (in_maps, kc, use_bias)."""
    bf = ml_dtypes.bfloat16
    scale = 1.0 / np.sqrt(D)
    use_bias = bool(np.any(bq) or np.any(bk) or np.any(bv))
    kc = 9 if use_bias else 8

    # S^T orientation: partition = key-in-chunk kk, free = query offset qq.
    # Valid (unmasked) iff qq >= kk.
    mka_np = np.where(np.arange(128)[None, :] >= np.arange(128)[:, None],
                      np.float32(0.0), np.float32(-1.0e30)).astype(bf)
    eye_np = np.eye(128, dtype=np.float32).astype(bf)

    def aug_x(xt):  # [1024, T] -> [kc*128, T]
        if not use_bias:
            return xt
        pad = np.zeros((128, xt.shape[1]), dtype=xt.dtype)
        pad[0, :] = 1.0
        return np.concatenate([xt, pad], axis=0)

    def aug_w(w, b):  # [1024, 512] -> [kc*128, 512]
        if not use_bias:
            return w
        pad = np.zeros((128, w.shape[1]), dtype=w.dtype)
        pad[0, :] = b
        return np.concatenate([w, pad], axis=0)

    in_maps = []
    for core in range(8):
        b = core // 2
        half = core % 2
        cs = slice(HALF * half, HALF * half + HALF)
        xt = np.ascontiguousarray(x[b, :T, :].T).astype(np.float32)

        def wlayout(w):  # [kc*128, 512] -> [4, 128, kc*128] partition-major
            r = w.reshape(kc, 128, 4, 128)       # k, p, m, c
            return np.ascontiguousarray(
                r.transpose(2, 1, 0, 3).reshape(4, 128, kc * 128))

        in_maps.append({
            "xT": aug_x(xt).astype(bf),
            "wq": wlayout(aug_w(Wq[:, cs] * scale, bq[cs] * scale)).astype(bf),
            "wk": wlayout(aug_w(Wk[:, cs], bk[cs])).astype(bf),
            "wv": wlayout(aug_w(Wv[:, cs], bv[cs])).astype(bf),
            "wp": Wp[cs, :].astype(bf),
            "mka": mka_np,
            "eye": eye_np,
        })
    return in_maps, kc, use_bias


def run(inputs: dict, T: int = 2048, trace: bool = False, tmpdir=None):
    """Returns (output [B,T,C] f32, BassKernelResults)."""
    from concourse.bass_utils import run_bass_kernel_spmd

    x = np.asarray(inputs["x"], dtype=np.float32)
    B = x.shape[0]
    in_maps, kc, _ = _prep_inputs(
        x, *[np.asarray(inputs[k], dtype=np.float32) for k in
             ("Wq", "bq", "Wk", "bk", "Wv", "bv", "Wp", "bp")], T)

    key = (kc, T)
    if key not in _NC_CACHE:
        _NC_CACHE[key] = _build_program(kc, T)
    nc = _NC_CACHE[key]

    res = run_bass_kernel_spmd(nc, in_maps, list(range(8)),
                               trace=trace, tmpdir=tmpdir)

    bp = np.asarray(inputs["bp"], dtype=np.float32)
    out = np.empty((B, T, C), dtype=np.float32)
    for b in range(B):
        acc = (res.results[2 * b]["outT"].astype(np.float32)
               + res.results[2 * b + 1]["outT"].astype(np.float32))
        out[b] = acc.T + bp[None, :]
    return out, res


def kernel(**inputs) -> np.ndarray:
    out, _ = run(inputs, T=2048, trace=False)
    return out
